# revision 1
# baseline (speedup 1.0000x reference)
"""Multi-head attention (B=4, N=2048, D=1024, H=16) on 8 TRN2 NeuronCores.

Sharding: 8 cores = batch(4) x sequence-half(2). Each core computes the full
attention output for its 1024-token slice of one batch (all 16 heads), so the
final unshard is a pure gather. The only cross-core traffic is an AllGather of
K^T and V between the two cores of each batch pair.

Per-core pipeline (bf16 matmul operands, fp32 PSUM accumulation):
  1. Cast x / w_qkv / w_proj to bf16, stage to DRAM, and DMA-transpose back so
     contraction dims sit on SBUF partitions.
  2. QKV projection. Q^T and K^T are produced in [d_out, token] orientation
     (lhsT = w_qkv^T tile, rhs = x^T); V in natural [token, d] orientation
     (lhsT = x^T tile, rhs = w_qkv^T).
  3. AllGather K^T then V across the pair (k-token axis spans both halves).
  4. Attention per head-pair p: S^T = (QK^T)^T via row-paired matmuls
     (contraction = head_dim 64, two heads in array row halves), exp on
     ScalarE straight out of PSUM (logits are bounded, no max subtraction),
     then O^T and the softmax denominator via col-paired matmuls over the
     k axis. The all-ones denominator lhsT replicates each head's denominator
     across its 64 output partitions, so normalization is a single full-width
     reciprocal + multiply on VectorE.
  5. Output projection from the accumulated attout^T tiles, bias add, DMA out.
"""

import sys

for _p in ("/opt/trn_rl_repo",):
    if _p not in sys.path:
        sys.path.insert(0, _p)

import numpy as np

import concourse.bass as bass
import concourse.mybir as mybir
import concourse.tile as tile
from concourse import bacc
from concourse.bass_utils import run_bass_kernel_spmd

B, N, D, H, HD = 4, 2048, 1024, 16, 64
SCALE = HD ** -0.5
NL = N // 2  # tokens per core
NCORES = 8
RG = [[0, 1], [2, 3], [4, 5], [6, 7]]
F32 = mybir.dt.float32
BF16 = mybir.dt.bfloat16
EXP = mybir.ActivationFunctionType.Exp


def _emit(tc, aps):
    nc = tc.nc
    x_l, wqkv, wproj, bias, out = (
        aps["x_local"], aps["w_qkv"], aps["w_proj"], aps["b_proj"], aps["out"])
    x_blk, wqkv_blk, wproj_blk = aps["x_blk"], aps["wqkv_blk"], aps["wproj_blk"]
    cc_k, cc_v, k_g, v_g = aps["cc_k"], aps["cc_v"], aps["k_g"], aps["v_g"]

    persist1 = tc.alloc_tile_pool(name="persist1", bufs=1)

    # ---- Phase A: load fp32, cast bf16, stage to DRAM column-blocked ------
    # (one [rows, 128] contiguous block per k-tile so the DMA-transposes
    # read contiguous DRAM). Loads on sync; fused blocked stores on scalar
    # (idle until the first exp). The sync queue carries ONLY prologue work
    # and transposes - every collective-gated DMA lives on scalar so the
    # in-order sync queue never blocks on a collective semaphore.
    prep = tc.alloc_tile_pool(name="prep", bufs=5)
    qkvp = tc.alloc_tile_pool(name="qkvp", bufs=1)

    def cast_tiles(src, blk, tiles):
        for i in tiles:
            t = prep.tile([128, D], F32, tag="ld_f32")
            nc.gpsimd.dma_start(out=t, in_=src[i * 128:(i + 1) * 128, :])
            tb = prep.tile([128, D], BF16, tag="cast_bf")
            nc.vector.tensor_copy(tb, t)
            dst = bass.AP(tensor=blk.tensor,
                          offset=blk.offset + i * 128 * 128,
                          ap=[[128, 128], [blk.ap[0][0], 8], [1, 128]])
            nc.scalar.dma_start(out=dst, in_=tb.rearrange("p (k c) -> p k c", k=8))

    # emission interleaves each group's loads with its transposes below

    bias_sb = persist1.tile([128, D], F32, tag="bias")
    bias_bcast = bass.AP(tensor=bias.tensor, offset=bias.offset,
                         ap=[[0, 128], *bias.ap])
    nc.scalar.dma_start(out=bias_sb, in_=bias_bcast)

    ones_sb = persist1.tile([128, 64], BF16, tag="ones")
    nc.vector.memset(ones_sb, 1.0)

    qT = [persist1.tile([128, NL], BF16, tag=f"qT{p}", name=f"qT{p}") for p in range(8)]
    kT = [persist1.tile([128, N], BF16, tag=f"kT{p}", name=f"kT{p}") for p in range(8)]
    vv = [persist1.tile([128, D], BF16, tag=f"v{kt}", name=f"v{kt}") for kt in range(16)]
    wpT_holder = [persist1.tile([128, D], BF16, tag=f"wpT{k}", name=f"wpT{k}")
                  for k in range(8)]


    xT = [qkvp.tile([128, NL], BF16, tag=f"xT{k}", name=f"xT{k}") for k in range(8)]
    wT = [qkvp.tile([128, 3 * D], BF16, tag=f"wT{k}", name=f"wT{k}") for k in range(8)]

    def wT_load(lo):
        for k in range(8):
            nc.sync.dma_start_transpose(
                out=wT[k][:, lo:lo + 1024], in_=wqkv_blk[k, lo:lo + 1024, :])

    # group-by-group: loads then the transposes that consume them, so the
    # in-order sync queue streams [loads | transposes] per group with no
    # cross-group blocking; all stores are on the scalar queue
    cast_tiles(x_l, x_blk, range(8))
    for k in range(8):
        nc.sync.dma_start_transpose(out=xT[k], in_=x_blk[k])
    cast_tiles(wqkv, wqkv_blk, range(8, 16))   # K rows 1024:2048
    wT_load(1024)
    cast_tiles(wqkv, wqkv_blk, range(16, 24))  # V rows 2048:3072
    wT_load(2048)
    cast_tiles(wqkv, wqkv_blk, range(0, 8))    # Q rows 0:1024
    wT_load(0)
    cast_tiles(wproj, wproj_blk, range(8))
    for k in range(8):
        nc.sync.dma_start_transpose(out=wpT_holder[k], in_=wproj_blk[k])

    with tc.tile_pool(name="qkvsb", bufs=2) as qkvsb, \
         tc.tile_pool(name="qkv_ps", bufs=2, space="PSUM") as qkvps:

        def proj_dT(m, dst_sb):
            ps = qkvps.tile([128, 2, 512], F32, tag="qkv_ps")
            for k in range(8):
                for qc in range(2):
                    nc.tensor.matmul(
                        out=ps[:, qc, :],
                        lhsT=wT[k][:, m * 128:(m + 1) * 128],
                        rhs=xT[k][:, qc * 512:(qc + 1) * 512],
                        start=(k == 0), stop=(k == 7))
            for qc in range(2):
                nc.vector.tensor_copy(dst_sb[:, qc * 512:(qc + 1) * 512], ps[:, qc, :])

        # K projection first so the K AllGather launches as early as possible
        for m in range(8, 16):
            ksb = qkvsb.tile([128, NL], BF16, tag="k_loc")
            proj_dT(m, ksb)
            nc.scalar.dma_start(out=cc_k[(m - 8) * 128:(m - 7) * 128, :], in_=ksb)
        nc.gpsimd.collective_compute(
            "AllGather", mybir.AluOpType.bypass, replica_groups=RG,
            ins=[cc_k], outs=[k_g])
        # gathered loads on scalar (its queue may block on the collective
        # semaphore without holding up any transpose)
        for p in range(8):
            nc.gpsimd.dma_start(out=kT[p][:, 0:NL], in_=k_g[0, p * 128:(p + 1) * 128, :])
            nc.gpsimd.dma_start(out=kT[p][:, NL:N], in_=k_g[1, p * 128:(p + 1) * 128, :])

        # V projection next so its AllGather overlaps the Q projection.
        # All remaining transposes are emitted BEFORE the ccV stores so the
        # in-order sync queue never parks a transpose behind a store that
        # waits on V-projection results.
        for t in range(8):
            vsb = qkvsb.tile([128, D], BF16, tag="v_loc")
            ps = qkvps.tile([128, 2, 512], F32, tag="qkv_ps")
            for k in range(8):
                for vc in range(2):
                    nc.tensor.matmul(
                        out=ps[:, vc, :],
                        lhsT=xT[k][:, t * 128:(t + 1) * 128],
                        rhs=wT[k][:, 2 * D + vc * 512:2 * D + (vc + 1) * 512],
                        start=(k == 0), stop=(k == 7))
            for vc in range(2):
                nc.vector.tensor_copy(vsb[:, vc * 512:(vc + 1) * 512], ps[:, vc, :])
            nc.scalar.dma_start(out=cc_v[t * 128:(t + 1) * 128, :], in_=vsb)
        nc.gpsimd.collective_compute(
            "AllGather", mybir.AluOpType.bypass, replica_groups=RG,
            ins=[cc_v], outs=[v_g])
        for kt in range(16):
            nc.gpsimd.dma_start(
                out=vv[kt], in_=v_g[kt // 8, (kt % 8) * 128:(kt % 8 + 1) * 128, :])

        # Q projection (overlaps the V gather; attention starts right after)
        for m in range(8):
            proj_dT(m, qT[m])

    qkvp.release()
    prep.release()

    # ---- Phase D: attention ----------------------------------------------
    persist2 = tc.alloc_tile_pool(name="persist2", bufs=1)
    attoutT = [persist2.tile([128, NL], BF16, tag=f"ao{p}", name=f"ao{p}") for p in range(8)]
    wpT = wpT_holder

    with tc.tile_pool(name="att_ps", bufs=2, space="PSUM") as attps, \
         tc.tile_pool(name="pT", bufs=4) as ppool, \
         tc.tile_pool(name="rc", bufs=2) as rpool:
        for p in range(8):
            for qc in range(2):
                o = attps.tile([128, 512], F32, tag="o_ps")
                dn = attps.tile([128, 512], F32, tag="den_ps")
                for kt in range(16):
                    s = attps.tile([128, 2, 512], F32, tag="s_ps")
                    for h in range(2):
                        nc.tensor.matmul(
                            out=s[:, h, :],
                            lhsT=kT[p][h * 64:(h + 1) * 64, kt * 128:(kt + 1) * 128],
                            rhs=qT[p][h * 64:(h + 1) * 64, qc * 512:(qc + 1) * 512],
                            start=True, stop=True,
                            tile_position=(h * 64, 0))
                    pt = ppool.tile([128, 2, 512], BF16, tag="pT")
                    nc.scalar.activation(pt, s, EXP, scale=SCALE)
                    for h in range(2):
                        nc.tensor.matmul(
                            out=o[h * 64:(h + 1) * 64, :],
                            lhsT=vv[kt][:, (2 * p + h) * 64:(2 * p + h + 1) * 64],
                            rhs=pt[:, h, :],
                            start=(kt == 0), stop=(kt == 15),
                            tile_position=(0, h * 64))
                    for h in range(2):
                        nc.tensor.matmul(
                            out=dn[h * 64:(h + 1) * 64, :],
                            lhsT=ones_sb,
                            rhs=pt[:, h, :],
                            start=(kt == 0), stop=(kt == 15),
                            tile_position=(0, h * 64))
                rc = rpool.tile([128, 512], F32, tag="rc")
                nc.vector.reciprocal(rc, dn)
                nc.vector.tensor_mul(attoutT[p][:, qc * 512:(qc + 1) * 512], o, rc)

    # ---- Phase E: output projection + bias --------------------------------
    with tc.tile_pool(name="proj_ps", bufs=2, space="PSUM") as projps, \
         tc.tile_pool(name="y_sb", bufs=3) as ypool:
        for tt in range(8):
            ps = projps.tile([128, 2, 512], F32, tag="proj_ps")
            for p in range(8):
                for ec in range(2):
                    nc.tensor.matmul(
                        out=ps[:, ec, :],
                        lhsT=attoutT[p][:, tt * 128:(tt + 1) * 128],
                        rhs=wpT[p][:, ec * 512:(ec + 1) * 512],
                        start=(p == 0), stop=(p == 7))
            yt = ypool.tile([128, D], F32, tag="y_sb")
            for ec in range(2):
                nc.vector.tensor_add(yt[:, ec * 512:(ec + 1) * 512], ps[:, ec, :],
                                     bias_sb[:, ec * 512:(ec + 1) * 512])
            nc.sync.dma_start(out=out[tt * 128:(tt + 1) * 128, :], in_=yt)
    persist2.release()
    persist1.release()


def _build():
    nc = bacc.Bacc("TRN2", target_bir_lowering=False, debug=False,
                   num_devices=NCORES)
    aps = {
        "x_local": nc.dram_tensor("x_local", [NL, D], F32, kind="ExternalInput").ap(),
        "w_qkv": nc.dram_tensor("w_qkv", [3 * D, D], F32, kind="ExternalInput").ap(),
        "w_proj": nc.dram_tensor("w_proj", [D, D], F32, kind="ExternalInput").ap(),
        "b_proj": nc.dram_tensor("b_proj", [D], F32, kind="ExternalInput").ap(),
        "out": nc.dram_tensor("out", [NL, D], F32, kind="ExternalOutput").ap(),
        "wqkv_blk": nc.dram_tensor("wqkv_blk", [8, 3 * D, 128], BF16).ap(),
        "wproj_blk": nc.dram_tensor("wproj_blk", [8, D, 128], BF16).ap(),
        "x_blk": nc.dram_tensor("x_blk", [8, NL, 128], BF16).ap(),
        "cc_k": nc.dram_tensor("cc_k", [D, NL], BF16).ap(),
        "cc_v": nc.dram_tensor("cc_v", [NL, D], BF16).ap(),
        "k_g": nc.dram_tensor("k_g", [2, D, NL], BF16).ap(),
        "v_g": nc.dram_tensor("v_g", [2, NL, D], BF16).ap(),
    }
    with tile.TileContext(nc) as tc:
        _emit(tc, aps)
    nc.compile()
    return nc


_NC = None


def _get_nc():
    global _NC
    if _NC is None:
        _NC = _build()
    return _NC


def run(x, w_qkv, w_proj, b_proj, **spmd_kwargs):
    nc = _get_nc()
    x = np.ascontiguousarray(np.asarray(x, dtype=np.float32))
    w_qkv = np.ascontiguousarray(np.asarray(w_qkv, dtype=np.float32))
    w_proj = np.ascontiguousarray(np.asarray(w_proj, dtype=np.float32))
    b_proj = np.ascontiguousarray(np.asarray(b_proj, dtype=np.float32))
    in_maps = []
    for c in range(NCORES):
        b, half = divmod(c, 2)
        in_maps.append({
            "x_local": np.ascontiguousarray(x[b, half * NL:(half + 1) * NL, :]),
            "w_qkv": w_qkv,
            "w_proj": w_proj,
            "b_proj": b_proj,
        })
    res = run_bass_kernel_spmd(nc, in_maps, list(range(NCORES)), **spmd_kwargs)
    y = np.empty((B, N, D), dtype=np.float32)
    for c in range(NCORES):
        b, half = divmod(c, 2)
        y[b, half * NL:(half + 1) * NL, :] = res.results[c]["out"]
    return y, res


def kernel(x, w_qkv, w_proj, b_proj):
    y, _ = run(x, w_qkv, w_proj, b_proj)
    return y



# revision 15
# speedup vs baseline: 1.2495x; 1.2495x over previous
"""Multi-head attention (B=4, N=2048, D=1024, H=16) on 8 TRN2 NeuronCores.

Sharding: 8 cores = batch(4) x sequence-half(2). Each core computes the full
attention output for its 1024-token slice of one batch (all 16 heads).

The only collective is a chunked AllGather of x^T (fp8) between the two cores
of each batch pair, launched right at the start: each core then computes K and
V for BOTH sequence halves locally (Q only for its own half). This removes the
mid-pipeline K/V AllGathers entirely; their latency hid poorly behind compute.

Compute recipe (fp8 where it pays, bf16 where precision needs it):
  - x cast to fp8 directly; w_qkv / w_proj scaled x32 into fp8 (keeps the
    0.02-scale weights well above the e4m3 subnormal cutoff). Scales are
    compensated in the exp() scale and the final bias add.
  - QKV projection in fp8 DoubleRow mode (2 contraction tiles per
    instruction): Q^T/K^T produced [d, tok] via lhsT = w^T tiles, V natural
    [tok, d] via lhsT = x^T tiles. Weight transposes are SBUF->SBUF DMA
    transposes of the bf16 casts - no DRAM staging round trip.
  - S^T per head pair via bf16 row-paired matmuls (contraction = head_dim 64),
    exp straight out of PSUM on ScalarE with fp8 output.
  - O^T via fp8 DoubleRow over k-token-tile pairs, with a ones column folded
    into each head's V tile so the softmax denominator accumulates for free
    into PSUM row 64 (removes the per-tile denominator matmuls of v1).
  - Normalization: reciprocal_approx_fast on the [1, 1024] denominator row,
    broadcast to 64 partitions with one tiny K=32 matmul against a
    zero-padded ones column, then a single VectorE multiply writing the
    normalized attention output as fp8 [64, 2heads, tok].
  - Output projection in fp8 DoubleRow pairing the two heads of each pair
    (contraction 2x64), interleaved into the attention sweep per q-chunk.
    Bias + descale fused in one scalar_tensor_tensor on GpSimd.
"""

import sys

for _p in ("/opt/trn_rl_repo",):
    if _p not in sys.path:
        sys.path.insert(0, _p)

import numpy as np

import concourse.bass as bass
import concourse.mybir as mybir
import concourse.tile as tile
from concourse import bacc
from concourse.bass_utils import run_bass_kernel_spmd

B, N, D, H, HD = 4, 2048, 1024, 16, 64
SCALE = HD ** -0.5
NL = N // 2  # tokens per core
NCORES = 8
RG = [[0, 1], [2, 3], [4, 5], [6, 7]]
F32 = mybir.dt.float32
BF16 = mybir.dt.bfloat16
FP8 = mybir.dt.float8e4
EXP = mybir.ActivationFunctionType.Exp
COPY = mybir.ActivationFunctionType.Copy
DR = mybir.MatmulPerfMode.DoubleRow
MUL = mybir.AluOpType.mult
ADD = mybir.AluOpType.add

WS = 32.0                      # fp8 scale on w_qkv, w_proj
EXPSCALE = SCALE / (WS * WS)   # Q and K each carry a factor WS
YSCALE = 1.0 / (WS * WS)       # attout (x32 via V) @ w_proj (x32)


def _emit(tc, aps):
    nc = tc.nc
    x_l, wqkv, wproj, bias, out = (
        aps["x_local"], aps["w_qkv"], aps["w_proj"], aps["b_proj"], aps["out"])
    cc_x, x_g = aps["cc_x"], aps["x_g"]

    pa = tc.alloc_tile_pool(name="persistA", bufs=1)
    pb = tc.alloc_tile_pool(name="persistB", bufs=1)

    # ---------------- persistent tiles ----------------
    bias_sb = pa.tile([128, D], F32, tag="bias")
    bias_bcast = bass.AP(tensor=bias.tensor, offset=bias.offset,
                         ap=[[0, 128], *bias.ap])
    nc.gpsimd.dma_start(out=bias_sb, in_=bias_bcast)

    ones32 = pa.tile([32, 64], BF16, tag="ones32")
    nc.vector.memset(ones32, 1.0)

    qT = [pa.tile([128, NL], BF16, tag=f"qT{m}", name=f"qT{m}") for m in range(8)]
    kT = [pa.tile([128, N], BF16, tag=f"kT{m}", name=f"kT{m}") for m in range(8)]
    # V with a ones column per head: [tok, ktpair-half, head, 64 V cols + 1]
    vv8 = [pa.tile([128, 2, 16, 65], FP8, tag=f"vv{t}", name=f"vv{t}")
           for t in range(8)]
    aoT = [pa.tile([64, 2, NL], FP8, tag=f"ao{p}", name=f"ao{p}") for p in range(8)]
    wp64 = [pa.tile([64, 2, D], FP8, tag=f"wp{p}", name=f"wp{p}") for p in range(8)]

    xT8 = pb.tile([128, 8, NL], FP8, tag="xT8", name="xT8")
    xg8 = pb.tile([128, 8, N], FP8, tag="xg8", name="xg8")
    wTq = pb.tile([128, 8, D], FP8, tag="wTq", name="wTq")
    wTk = pb.tile([128, 8, D], FP8, tag="wTk", name="wTk")
    wTv = pb.tile([128, 8, D], FP8, tag="wTv", name="wTv")
    wpT8 = pb.tile([128, 8, D], FP8, tag="wpT8", name="wpT8")

    # ones columns of vv8 (written once; V copies fill cols 0:64)
    for t in range(8):
        nc.vector.memset(vv8[t][:, :, :, 64:65], 1.0)

    # ---------------- prologue: x -> xT8, cc chunks, xg8 ----------------
    xtp = tc.alloc_tile_pool(name="xtp", bufs=1)
    prep = tc.alloc_tile_pool(name="prep", bufs=4)
    castp = tc.alloc_tile_pool(name="castp", bufs=3)
    wtp = tc.alloc_tile_pool(name="wtp", bufs=3)

    xTb = xtp.tile([128, 8, NL], BF16, tag="xTb", name="xTb")
    for t in range(8):
        xf = prep.tile([128, D], F32, tag="ld_f32")
        nc.scalar.dma_start(out=xf, in_=x_l[t * 128:(t + 1) * 128, :])
        xb = castp.tile([128, D], BF16, tag="cast_bf")
        nc.vector.tensor_copy(xb, xf)
        nc.sync.dma_start_transpose(
            out=xTb[:, :, t * 128:(t + 1) * 128], in_=xb)

    # per k-pair: cast fp8, store cc chunk, launch collective
    for j in range(4):
        nc.vector.tensor_copy(xT8[:, 2 * j:2 * j + 2, :],
                              xTb[:, 2 * j:2 * j + 2, :])
        nc.scalar.dma_start(
            out=cc_x[2 * j * 128:(2 * j + 2) * 128, :].rearrange(
                "(a p) b -> p a b", p=128),
            in_=xT8[:, 2 * j:2 * j + 2, :])
    for j in range(4):
        nc.gpsimd.collective_compute(
            "AllGather", mybir.AluOpType.bypass, replica_groups=RG,
            ins=[cc_x[2 * j * 128:(2 * j + 2) * 128, :]],
            outs=[x_g[j]])
    for j in range(4):
        for half in range(2):
            nc.gpsimd.dma_start(
                out=xg8[:, 2 * j:2 * j + 2, half * NL:(half + 1) * NL],
                in_=x_g[j, half].rearrange("(a p) b -> p a b", p=128))

    # ---------------- prologue: weights ----------------
    # Per section: batch the 8 f32 loads on one queue, then per-row
    # scale-cast (scalar) -> SBUF->SBUF transpose (sync) -> fp8 copy (vector).
    def w_section(src_rows, dst8, queue):
        wfs = [prep.tile([128, D], F32, tag="ld_f32", name=f"wf{r}")
               for r in range(8)]
        for r in range(8):
            queue.dma_start(out=wfs[r], in_=src_rows[r * 128:(r + 1) * 128, :])
        for r in range(8):
            wb = castp.tile([128, D], BF16, tag="cast_bf")
            nc.scalar.activation(wb, wfs[r], COPY, scale=WS)
            wt = wtp.tile([128, 8, 128], BF16, tag="wt")
            nc.sync.dma_start_transpose(out=wt, in_=wb)
            nc.vector.tensor_copy(dst8[:, :, r * 128:(r + 1) * 128], wt)

    w_section(wqkv, wTq, nc.sync)                  # Q rows 0:1024
    w_section(wqkv[D:2 * D, :], wTk, nc.scalar)    # K rows 1024:2048
    w_section(wqkv[2 * D:3 * D, :], wTv, nc.scalar)  # V rows 2048:3072
    w_section(wproj, wpT8, nc.gpsimd)
    # repack w_proj^T into per-pair half-partition tiles [64, 2, D]
    for p in range(8):
        for h in range(2):
            nc.gpsimd.dma_start(
                out=wp64[p][:, h, :],
                in_=wpT8[h * 64:(h + 1) * 64, p, :])

    # ---------------- QKV projection (fp8 DoubleRow) ----------------
    with tc.tile_pool(name="qkv_ps", bufs=2, space="PSUM") as qkvps:
        # Q first: depends only on local xT8, overlaps the x collective
        for m in range(8):
            ps = qkvps.tile([128, 2, 512], F32, tag="qkv_ps")
            for j in range(4):
                for qc in range(2):
                    nc.tensor.matmul(
                        out=ps[:, qc, :],
                        lhsT=wTq[:, 2 * j:2 * j + 2, m * 128:(m + 1) * 128],
                        rhs=xT8[:, 2 * j:2 * j + 2, qc * 512:(qc + 1) * 512],
                        start=(j == 0), stop=(j == 3), perf_mode=DR)
            nc.vector.tensor_copy(qT[m].rearrange("p (a b) -> p a b", a=2), ps)

        # K for both halves
        for m in range(8):
            for k2 in range(2):
                ps = qkvps.tile([128, 2, 512], F32, tag="qkv_ps")
                for j in range(4):
                    for kc in range(2):
                        c = k2 * 2 + kc
                        nc.tensor.matmul(
                            out=ps[:, kc, :],
                            lhsT=wTk[:, 2 * j:2 * j + 2, m * 128:(m + 1) * 128],
                            rhs=xg8[:, 2 * j:2 * j + 2, c * 512:(c + 1) * 512],
                            start=(j == 0), stop=(j == 3), perf_mode=DR)
                nc.vector.tensor_copy(
                    kT[m][:, k2 * 1024:(k2 + 1) * 1024].rearrange(
                        "p (a b) -> p a b", a=2), ps)

        # V for both halves, natural [tok, d] orientation
        for t in range(16):
            ps = qkvps.tile([128, 2, 512], F32, tag="qkv_ps")
            for j in range(4):
                for vc in range(2):
                    nc.tensor.matmul(
                        out=ps[:, vc, :],
                        lhsT=xg8[:, 2 * j:2 * j + 2, t * 128:(t + 1) * 128],
                        rhs=wTv[:, 2 * j:2 * j + 2, vc * 512:(vc + 1) * 512],
                        start=(j == 0), stop=(j == 3), perf_mode=DR)
            nc.vector.tensor_copy(
                vv8[t // 2][:, t % 2, :, 0:64],
                ps.rearrange("p a (b c) -> p (a b) c", b=8))

    wtp.release()
    castp.release()
    prep.release()
    xtp.release()
    pb.release()

    # ---------------- attention + interleaved output projection ----------
    ypool = tc.alloc_tile_pool(name="y_sb", bufs=2)
    with tc.tile_pool(name="att_ps", bufs=2, space="PSUM") as spool, \
         tc.tile_pool(name="o_ps", bufs=2, space="PSUM") as opool, \
         tc.tile_pool(name="pt", bufs=2) as ptpool, \
         tc.tile_pool(name="rd", bufs=2) as rdpool:

        pending_fin = []

        def flush_fin():
            while pending_fin:
                pending_fin.pop(0)()

        it = 0
        for qc in range(2):
            for p in range(8):
                o = opool.tile([65, 2, 512], F32, tag="o_ps")
                pt = None
                for kt in range(16):
                    s = spool.tile([128, 2, 512], F32, tag="s_ps")
                    for h in range(2):
                        nc.tensor.matmul(
                            out=s[:, h, :],
                            lhsT=kT[p][h * 64:(h + 1) * 64,
                                       kt * 128:(kt + 1) * 128],
                            rhs=qT[p][h * 64:(h + 1) * 64,
                                      qc * 512:(qc + 1) * 512],
                            start=True, stop=True,
                            tile_position=(h * 64, 0))
                    if kt == 2:
                        flush_fin()  # previous iteration's finalize, late
                    if kt % 2 == 0:
                        pt = ptpool.tile([128, 2, 2, 512], FP8, tag="pt")
                    nc.scalar.activation(pt[:, kt % 2, :, :], s, EXP,
                                         scale=EXPSCALE)
                    if kt % 2 == 1:
                        for h in range(2):
                            nc.tensor.matmul(
                                out=o[:, h, :],
                                lhsT=vv8[kt // 2][:, :, 2 * p + h, :],
                                rhs=pt[:, :, h, :],
                                start=(kt == 1), stop=(kt == 15),
                                perf_mode=DR)

                def fin(o=o, p=p, qc=qc, it=it):
                    rd32 = rdpool.tile([1, 2, 512], F32, tag="rd32")
                    nc.vector.reciprocal_approx_fast(out=rd32,
                                                     in_=o[64:65, :, :])
                    rd16 = rdpool.tile([32, 2, 512], BF16, tag="rd16")
                    if it < 2:  # zero the pad rows once per pool buffer
                        nc.vector.memset(rd16, 0.0)
                    nc.vector.tensor_copy(rd16[0:1, :, :], rd32)
                    rdb = spool.tile([128, 2, 512], F32, tag="s_ps")
                    for c in range(2):
                        nc.tensor.matmul(
                            out=rdb[0:64, c, :], lhsT=ones32,
                            rhs=rd16[:, c, :], start=True, stop=True)
                    rdsb = rdpool.tile([64, 2, 512], BF16, tag="rdsb")
                    nc.vector.tensor_copy(rdsb, rdb[0:64, :, :])
                    nc.vector.tensor_mul(
                        aoT[p][:, :, qc * 512:(qc + 1) * 512],
                        o[0:64, :, :], rdsb)

                pending_fin.append(fin)
                it += 1

            flush_fin()
            # output projection for the two 128-token tiles of this q-chunk
            for tt in range(2 * qc, 2 * qc + 2):
                ps = spool.tile([128, 2, 512], F32, tag="s_ps")
                for p in range(8):
                    for ec in range(2):
                        nc.tensor.matmul(
                            out=ps[:, ec, :],
                            lhsT=aoT[p][:, :, tt * 128:(tt + 1) * 128],
                            rhs=wp64[p][:, :, ec * 512:(ec + 1) * 512],
                            start=(p == 0), stop=(p == 7), perf_mode=DR)
                yt = ypool.tile([128, D], F32, tag="y_sb")
                nc.vector.scalar_tensor_tensor(
                    out=yt.rearrange("p (a b) -> p a b", a=2), in0=ps,
                    scalar=YSCALE, in1=bias_sb.rearrange(
                        "p (a b) -> p a b", a=2),
                    op0=MUL, op1=ADD)
                nc.sync.dma_start(out=out[tt * 128:(tt + 1) * 128, :], in_=yt)

    ypool.release()
    pa.release()


def _build():
    nc = bacc.Bacc("TRN2", target_bir_lowering=False, debug=False,
                   num_devices=NCORES)
    aps = {
        "x_local": nc.dram_tensor("x_local", [NL, D], F32,
                                  kind="ExternalInput").ap(),
        "w_qkv": nc.dram_tensor("w_qkv", [3 * D, D], F32,
                                kind="ExternalInput").ap(),
        "w_proj": nc.dram_tensor("w_proj", [D, D], F32,
                                 kind="ExternalInput").ap(),
        "b_proj": nc.dram_tensor("b_proj", [D], F32,
                                 kind="ExternalInput").ap(),
        "out": nc.dram_tensor("out", [NL, D], F32, kind="ExternalOutput").ap(),
        "cc_x": nc.dram_tensor("cc_x", [D, NL], FP8).ap(),
        "x_g": nc.dram_tensor("x_g", [4, 2, 256, NL], FP8).ap(),
    }
    with tile.TileContext(nc) as tc:
        _emit(tc, aps)
    nc.compile()
    return nc


_NC = None


def _get_nc():
    global _NC
    if _NC is None:
        _NC = _build()
    return _NC


def run(x, w_qkv, w_proj, b_proj, **spmd_kwargs):
    nc = _get_nc()
    x = np.ascontiguousarray(np.asarray(x, dtype=np.float32))
    w_qkv = np.ascontiguousarray(np.asarray(w_qkv, dtype=np.float32))
    w_proj = np.ascontiguousarray(np.asarray(w_proj, dtype=np.float32))
    b_proj = np.ascontiguousarray(np.asarray(b_proj, dtype=np.float32))
    in_maps = []
    for c in range(NCORES):
        b, half = divmod(c, 2)
        in_maps.append({
            "x_local": np.ascontiguousarray(x[b, half * NL:(half + 1) * NL, :]),
            "w_qkv": w_qkv,
            "w_proj": w_proj,
            "b_proj": b_proj,
        })
    res = run_bass_kernel_spmd(nc, in_maps, list(range(NCORES)), **spmd_kwargs)
    y = np.empty((B, N, D), dtype=np.float32)
    for c in range(NCORES):
        b, half = divmod(c, 2)
        y[b, half * NL:(half + 1) * NL, :] = res.results[c]["out"]
    return y, res


def kernel(x, w_qkv, w_proj, b_proj):
    y, _ = run(x, w_qkv, w_proj, b_proj)
    return y
